# revision 13
# baseline (speedup 1.0000x reference)
"""GCN (GCNConv + ReLU) message-passing kernel for 8 Trainium2 NeuronCores.

v3 strategy (dst-sharded graph parallelism, pipelined AllGather, chunked
gather stream):
  - Nodes sharded contiguously across 8 cores (12500 each, padded 12544).
  - The local node range is split into K=4 block-aligned slices. Phase A
    computes h'_c = dinv_c * (x_c @ W) (PE matmul, bf16 x staged on host,
    4 dst blocks per x DMA), writing each slice to its own internal DRAM
    buffer cc_in_j (fp16). As soon as slice j is written, AllGather_j
    broadcasts it: cc_out_j = concat over cores of slice j.
  - Edge srcs fall into the K "groups" (one per slice). Per core, edges
    are bucketed by (src group g, dst block b) and laid out in
    (g, super, b) order with sections packed tightly (padded only to the
    cross-core max per section and to 128 slots per (g, super) run).
  - Phase C runs group-major: gathers + segment-sum for group g only need
    AllGather_j=g, so groups 1..3 overlap with outstanding collectives.
  - Per-edge messages fetched with SWDGE dma_gather (int16 indices into
    cc_out_g), one gather call per 32-tile chunk (4096 edges) so the
    GpSimd descriptor-gen fixed cost is amortized and matmuls for chunk c
    can start while chunk c+1 is still gathering. Segment-sum via
    TensorE: one-hot selectors (fp16 is_equal of dst offsets vs
    per-section-shifted iota) contracted with fp16 message tiles in PSUM;
    per section, psum*dinv[b] is accumulated into an SBUF f32 accumulator
    preloaded with dinv^2*h + b (self loop).
  - Epilogue: ReLU(acc) -> out.

Host-side work is limited to integer index preprocessing (edge bucketing,
degree counts) and layout/dtype staging; all floating-point math runs on
device.
"""

import math
import sys

import numpy as np

sys.path.insert(0, "/opt/trn_rl_repo")

import ml_dtypes  # noqa: E402

import concourse.bass as bass  # noqa: E402
import concourse.bacc as bacc  # noqa: E402
import concourse.mybir as mybir  # noqa: E402
from concourse import tile  # noqa: E402
from concourse.bass_utils import run_bass_kernel_spmd  # noqa: E402

BF16 = ml_dtypes.bfloat16
FP16 = np.float16

# ----- problem constants (hardcoded; kernel.py must be self-contained) -----
N_NODES = 100000
D_IN = 256
D_OUT = 128
N_CORES = 8

# test-harness hooks (harness leaves these at defaults)
TRACE = False
LAST_RES = None


class Cfg:
    """Static, per-compile configuration (identical across cores)."""

    def __init__(self, n_nodes, d_in, d_out, n_cores, n_slices=4,
                 blocks_per_super=24):
        assert n_nodes % n_cores == 0
        self.n_nodes = n_nodes
        self.d_in = d_in
        self.d_out = d_out
        assert d_out == 128, "kernel assumes 128 output features"
        assert d_in % 128 == 0
        self.kchunks = d_in // 128
        self.n_cores = n_cores
        self.ns = n_nodes // n_cores          # nodes per core
        self.nb = math.ceil(self.ns / 128)    # dst blocks per core
        self.ns_pad = self.nb * 128
        self.bs = blocks_per_super
        self.nsup = math.ceil(self.nb / self.bs)
        # K block-aligned slices of the local node range
        K = min(n_slices, self.nb)
        # bathtub slice sizes: small first slice so AllGather0 fires early,
        # small last slice so the AG3-dependent tail is short
        # (each slice <=31 blocks for int16 gather indices)
        if self.nb == 98 and K == 4:
            self.slice_blocks = [8, 31, 31, 28]
        else:
            s0 = max(1, self.nb // (2 * K))
            base, rem = divmod(self.nb - s0, K - 1) if K > 1 else (0, 0)
            self.slice_blocks = [s0] + [base + (1 if j < rem else 0)
                                        for j in range(K - 1)]
        self.K = K
        self.slice_b0 = np.concatenate(
            [[0], np.cumsum(self.slice_blocks)]).astype(np.int64)
        self.slice_rows = [bl * 128 for bl in self.slice_blocks]
        # gather-group g corresponds to slice g of EVERY core:
        # cc_out_g rows = concat_c cc_in_g(core c), R_g = n_cores*rows_g
        self.group_rows = [n_cores * r for r in self.slice_rows]
        assert max(self.group_rows) <= 32767, "int16 gather index overflow"
        # filled by preprocessing:
        self.Lsec = None          # [K][nb] cross-core-max section sizes
        self.run_tiles = None     # [K][nsup] padded tiles per run
        self.run_slot0 = None     # [K][nsup] global slot offset
        self.sec_off = None       # {(g,b): offset of section inside its run}
        self.tot_slots = None

    def blocks_of_super(self, s):
        return range(s * self.bs, min((s + 1) * self.bs, self.nb))

    def slice_of_block(self, b):
        return int(np.searchsorted(self.slice_b0, b, side="right") - 1)


def preprocess(x, edge_index, W, b, cfg: Cfg):
    """Integer/layout-only host prep. Returns per-core input dicts."""
    n, ns, K = cfg.n_nodes, cfg.ns, cfg.K
    src = np.asarray(edge_index[0], dtype=np.int64)
    dst = np.asarray(edge_index[1], dtype=np.int64)
    x = np.asarray(x, dtype=np.float32)
    W = np.asarray(W, dtype=np.float32)
    b = np.asarray(b, dtype=np.float32)

    # map src global node -> (group, within-group row)
    s_core = src // ns
    s_loc = src % ns
    s_blk = s_loc // 128
    s_g = np.searchsorted(cfg.slice_b0, s_blk, side="right") - 1
    r0 = np.array([cfg.slice_b0[j] * 128 for j in range(K)])
    rows = np.array(cfg.slice_rows)
    s_idx = s_core * rows[s_g] + (s_loc - r0[s_g])  # within-group row

    core_of = dst // ns
    order = np.argsort(core_of, kind="stable")
    src_o, dst_o, sg_o, sidx_o = src[order], dst[order], s_g[order], s_idx[order]
    core_bounds = np.searchsorted(core_of[order], np.arange(cfg.n_cores + 1))

    percore = []
    counts = np.zeros((cfg.n_cores, K, cfg.nb), dtype=np.int64)
    for c in range(cfg.n_cores):
        lo, hi = core_bounds[c], core_bounds[c + 1]
        d_c = dst_o[lo:hi] - c * ns
        g_c = sg_o[lo:hi]
        i_c = sidx_o[lo:hi]
        blk = d_c // 128
        key = g_c * cfg.nb + blk
        o = np.argsort(key, kind="stable")
        d_c, g_c, i_c, key = d_c[o], g_c[o], i_c[o], key[o]
        counts[c] = np.bincount(key, minlength=K * cfg.nb).reshape(K, cfg.nb)
        deg = np.bincount(d_c, minlength=ns) + 1  # + self loop
        percore.append({"d": d_c, "g": g_c, "i": i_c, "key": key, "deg": deg})

    # cross-core-uniform (unrounded) section sizes
    Lsec = counts.max(axis=0)  # [K, nb]
    cfg.Lsec = Lsec

    # run layout: for g, for super s: sections (g, b in super) tight,
    # run padded to a multiple of 128 slots
    run_tiles = np.zeros((K, cfg.nsup), dtype=np.int64)
    run_slot0 = np.zeros((K, cfg.nsup), dtype=np.int64)
    sec_off = {}
    off = 0
    for g in range(K):
        for s in range(cfg.nsup):
            run_slot0[g, s] = off
            roff = 0
            for bb in cfg.blocks_of_super(s):
                sec_off[(g, bb)] = roff
                roff += int(Lsec[g, bb])
            rt = (roff + 127) // 128
            run_tiles[g, s] = rt
            off += rt * 128
    tot_slots = off
    cfg.run_tiles = run_tiles
    cfg.run_slot0 = run_slot0
    cfg.sec_off = sec_off
    cfg.tot_slots = tot_slots

    # verify <=4 sections per tile and no iota-offset collision within a tile
    for g in range(K):
        for s in range(cfg.nsup):
            bounds = []
            for bb in cfg.blocks_of_super(s):
                if Lsec[g, bb] > 0:
                    o0 = sec_off[(g, bb)]
                    bounds.append((o0, o0 + int(Lsec[g, bb]), bb - s * cfg.bs))
            for t in range(int(run_tiles[g, s])):
                lo, hi = t * 128, (t + 1) * 128
                ords = [o % 4 for (a, bnd, o) in bounds if a < hi and bnd > lo]
                assert len(ords) == len(set(ords)), "iota offset collision"

    in_maps = []
    for c in range(cfg.n_cores):
        pc = percore[c]
        idx_all = np.zeros(tot_slots, dtype=np.int16)
        dst_all = np.full(tot_slots, -1.0, dtype=np.float32)
        cnt = np.bincount(pc["key"], minlength=K * cfg.nb).reshape(K, cfg.nb)
        flat = 0
        for g in range(K):
            for bb in range(cfg.nb):
                m = int(cnt[g, bb])
                if m:
                    sl = slice(flat, flat + m)
                    s = bb // cfg.bs
                    o0 = int(cfg.run_slot0[g, s]) + cfg.sec_off[(g, bb)]
                    idx_all[o0:o0 + m] = pc["i"][sl].astype(np.int16)
                    oshift = 128.0 * ((bb - s * cfg.bs) % 4)
                    dst_all[o0:o0 + m] = (
                        pc["d"][sl] - bb * 128).astype(np.float32) + oshift
                flat += m
        # wrap idx into 16 partitions, replicated to 128
        idx_w16 = idx_all.reshape(-1, 16).T.copy()          # [16, tot/16]
        idx_w = np.tile(idx_w16, (8, 1))                     # [128, tot/16]
        dst_w = np.ascontiguousarray(
            dst_all.reshape(-1, 128).T).astype(FP16)         # [128, tot/128]

        deg_pad = np.ones(cfg.ns_pad, dtype=np.int32)
        deg_pad[:ns] = pc["deg"].astype(np.int32)
        deg_w = np.ascontiguousarray(deg_pad.reshape(cfg.nb, 128).T)  # [128, nb]

        xs = x[c * ns:(c + 1) * ns]
        xT = np.zeros((cfg.d_in, cfg.ns_pad), dtype=BF16)
        xT[:, :ns] = xs.T.astype(BF16)

        in_maps.append({
            "xT": xT,
            "Wm": W.copy(),
            "bias": b.reshape(1, -1).copy(),
            "deg": deg_w,
            "idx": idx_w,
            "dstloc": dst_w,
        })
    return in_maps, tot_slots


def build_program(cfg: Cfg, mock_cc=False, gchunk=8, dds=16384, nq=4):
    """Builds the SPMD bass program (same NEFF on every core)."""
    fp32 = mybir.dt.float32
    bf16 = mybir.dt.bfloat16
    fp16 = mybir.dt.float16
    K = cfg.K
    DO = cfg.d_out
    tot_slots = cfg.tot_slots
    Lsec = cfg.Lsec

    nc = bacc.Bacc("TRN2", target_bir_lowering=False,
                   num_devices=cfg.n_cores, debug=False,
                   dynamic_dma_scratch_size=dds,
                   num_swdge_queues=nq)

    xT = nc.dram_tensor("xT", [cfg.d_in, cfg.ns_pad], bf16, kind="ExternalInput")
    Wm = nc.dram_tensor("Wm", [cfg.d_in, DO], fp32, kind="ExternalInput")
    bias = nc.dram_tensor("bias", [1, DO], fp32, kind="ExternalInput")
    deg = nc.dram_tensor("deg", [128, cfg.nb], mybir.dt.int32, kind="ExternalInput")
    idx = nc.dram_tensor("idx", [128, tot_slots // 16], mybir.dt.int16,
                         kind="ExternalInput")
    dstloc = nc.dram_tensor("dstloc", [128, tot_slots // 128], fp16,
                            kind="ExternalInput")
    out = nc.dram_tensor("out", [cfg.ns, DO], fp32, kind="ExternalOutput")

    cc_in = [nc.dram_tensor(f"cc_in{j}", [cfg.slice_rows[j], DO], bf16)
             for j in range(K)]
    cc_out = [nc.dram_tensor(f"cc_out{j}", [cfg.group_rows[j], DO], bf16,
                             addr_space="Shared")
              for j in range(K)]

    # max section span in tiles (for sel tile sizing)
    max_span = 1
    for g in range(K):
        for bb in range(cfg.nb):
            ln = int(Lsec[g, bb])
            if ln == 0:
                continue
            o0 = cfg.sec_off[(g, bb)]
            max_span = max(max_span, (o0 + ln - 1) // 128 - o0 // 128 + 1)
    smax = max_span
    max_run_tiles = int(cfg.run_tiles.max())

    from contextlib import ExitStack
    with tile.TileContext(nc) as tc, ExitStack() as ctx:
        const = ctx.enter_context(tc.tile_pool(name="const", bufs=1))
        xpool = ctx.enter_context(tc.tile_pool(name="x", bufs=3))
        hpsum = ctx.enter_context(tc.tile_pool(name="hpsum", bufs=4, space="PSUM"))
        htmp = ctx.enter_context(tc.tile_pool(name="htmp", bufs=4))
        msgp = ctx.enter_context(tc.tile_pool(name="msg", bufs=8))
        idxp = ctx.enter_context(tc.tile_pool(name="idx", bufs=3))
        selp = ctx.enter_context(tc.tile_pool(name="sel", bufs=4))
        pspool = ctx.enter_context(tc.tile_pool(name="ps", bufs=4, space="PSUM"))
        epool = ctx.enter_context(tc.tile_pool(name="ep", bufs=4))

        # ---------------- phase 0: constants ----------------
        W_f = const.tile([128, cfg.kchunks, DO], fp32)
        for k in range(cfg.kchunks):
            nc.sync.dma_start(W_f[:, k, :], Wm[k * 128:(k + 1) * 128, :])
        W_b = const.tile([128, cfg.kchunks, DO], bf16)
        nc.vector.tensor_copy(W_b[:, :, :], W_f[:, :, :])

        deg_i = const.tile([128, cfg.nb], mybir.dt.int32)
        nc.sync.dma_start(deg_i[:, :], deg[:, :])
        deg_f = const.tile([128, cfg.nb], fp32)
        nc.vector.tensor_copy(deg_f[:, :], deg_i[:, :])
        deg_sq = const.tile([128, cfg.nb], fp32)
        nc.scalar.activation(deg_sq[:, :], deg_f[:, :],
                             mybir.ActivationFunctionType.Sqrt)
        dinv = const.tile([128, cfg.nb], fp32)
        nc.vector.reciprocal(dinv[:, :], deg_sq[:, :])
        dinv2 = const.tile([128, cfg.nb], fp32)
        nc.vector.tensor_tensor(dinv2[:, :], dinv[:, :], dinv[:, :],
                                mybir.AluOpType.mult)

        b_row = const.tile([1, DO], fp32)
        nc.sync.dma_start(b_row[:, :], bias[:, :])
        ones_row = const.tile([1, 128], fp32)
        nc.vector.memset(ones_row[:, :], 1.0)
        bt_ps = pspool.tile([128, DO], fp32, tag="ps")
        nc.tensor.matmul(bt_ps[:, :], ones_row[:, :], b_row[:, :],
                         start=True, stop=True)
        b_tile = const.tile([128, DO], fp32)
        nc.vector.tensor_copy(b_tile[:, :], bt_ps[:, :])

        # 4 iota variants (j + 128*k), fp16
        iota4 = const.tile([128, 4, 128], fp16)
        nc.gpsimd.iota(iota4[:, 0, :], [[1, 128]], channel_multiplier=0,
                       allow_small_or_imprecise_dtypes=True)
        for k in range(1, 4):
            nc.vector.tensor_scalar(iota4[:, k, :], iota4[:, 0, :],
                                    float(128 * k), None,
                                    mybir.AluOpType.add)

        dst_sb = const.tile([128, tot_slots // 128], fp16)

        # SBUF f32 accumulator; phase A preloads dinv^2*h + b (self loop)
        acc = const.tile([128, cfg.nb, DO], fp32)

        # ---------------- phase A + pipelined AllGathers ----------------
        _sidA, _ = nc.enter_named_scope("phA", False)
        for j in range(K):
            b_lo, b_hi = int(cfg.slice_b0[j]), int(cfg.slice_b0[j + 1])
            for k0 in range(b_lo, b_hi, 4):
                kw = min(4, b_hi - k0)
                xb = xpool.tile([128, cfg.kchunks, 4 * 128], bf16, tag="xb")
                nc.sync.dma_start(
                    xb[:, :, :kw * 128],
                    xT[:, k0 * 128:(k0 + kw) * 128].rearrange(
                        "(c p) n -> p c n", p=128))
                for k in range(k0, k0 + kw):
                    co = (k - k0) * 128
                    ph = hpsum.tile([128, DO], fp32, tag="hps")
                    for kb in range(cfg.kchunks):
                        nc.tensor.matmul(ph[:, :], xb[:, kb, co:co + 128],
                                         W_b[:, kb, :], start=(kb == 0),
                                         stop=(kb == cfg.kchunks - 1))
                    # self-loop + bias term preloaded into the accumulator
                    nc.vector.scalar_tensor_tensor(
                        acc[:, k, :], ph[:, :], dinv2[:, k:k + 1], b_tile[:, :],
                        mybir.AluOpType.mult, mybir.AluOpType.add)
                    # h' (fp16) -> cc_in[j]; scale on the (idle) scalar
                    # engine so phC selector builds don't stall phase A
                    hb = htmp.tile([128, DO], bf16, tag="hb")
                    nc.scalar.activation(hb[:, :], ph[:, :],
                                         mybir.ActivationFunctionType.Copy,
                                         scale=dinv[:, k:k + 1])
                    kr = k * 128 - b_lo * 128
                    nc.sync.dma_start(cc_in[j][kr:kr + 128, :], hb[:, :])
        nc.leave_named_scope("phA", _sidA, False)

        # phC-only constant, loaded behind phase A's x stream
        nc.sync.dma_start(dst_sb[:, :], dstloc[:, :])

        # AllGather triggers, emitted back-to-back so later groups'
        # collectives overlap with earlier groups' gathers/compute.
        _sidB, _ = nc.enter_named_scope("allgather", False)
        for j in range(K):
            if cfg.n_cores > 1 and not mock_cc:
                nc.gpsimd.collective_compute(
                    "AllGather",
                    mybir.AluOpType.bypass,
                    replica_groups=[list(range(cfg.n_cores))],
                    ins=[cc_in[j][:, :]],
                    outs=[cc_out[j][:, :]],
                )
            else:
                for cpy in range(cfg.n_cores):
                    r = cfg.slice_rows[j]
                    nc.sync.dma_start(cc_out[j][cpy * r:(cpy + 1) * r, :],
                                      cc_in[j][:, :])
        nc.leave_named_scope("allgather", _sidB, False)

        # ---------------- phase C: gather + segment-sum + epilogue -------
        _sidC, _ = nc.enter_named_scope("phC", False)
        gidx = [0]
        last_g = {}
        for bb in range(cfg.nb):
            lg = -1
            for g in range(K):
                if int(Lsec[g, bb]) > 0:
                    lg = g
            last_g[bb] = lg
        for bb in range(cfg.nb):
            if last_g[bb] < 0:
                t2 = epool.tile([128, DO], fp32, tag="t2")
                nc.scalar.activation(t2[:, :], acc[:, bb, :],
                                     mybir.ActivationFunctionType.Relu)
                rows = min(128, cfg.ns - bb * 128)
                nc.sync.dma_start(out[bb * 128:bb * 128 + rows, :],
                                  t2[:rows, :])

        for g in range(K):
            for s in range(cfg.nsup):
                ntiles = int(cfg.run_tiles[g, s])
                if ntiles == 0:
                    continue
                slot0 = int(cfg.run_slot0[g, s])
                idx_t = idxp.tile([128, max_run_tiles * 8], mybir.dt.int16,
                                  tag="idx")
                nc.sync.dma_start(idx_t[:, :ntiles * 8],
                                  idx[:, slot0 // 16:
                                      (slot0 + ntiles * 128) // 16])
                # section list for this run: (bb, t_lo, t_hi, kvar)
                secs = []
                for bb in cfg.blocks_of_super(s):
                    ln = int(Lsec[g, bb])
                    if ln == 0:
                        continue
                    o0 = cfg.sec_off[(g, bb)]
                    secs.append((bb, o0 // 128, (o0 + ln - 1) // 128,
                                 (bb - s * cfg.bs) % 4))
                nchunks = (ntiles + gchunk - 1) // gchunk
                msg_tiles = [None] * nchunks
                si = 0
                for c in range(nchunks):
                    c0 = c * gchunk
                    cnt = min(gchunk, ntiles - c0)
                    mt = msgp.tile([128, gchunk, 128], bf16, name="msg_ch",
                                   tag="msg")
                    msg_tiles[c] = mt
                    n_sg = cnt * 128
                    nc.gpsimd.dma_gather(
                        mt[:, :cnt, :],
                        cc_out[g][:, :],
                        idx_t[:, c0 * 8:c0 * 8 + n_sg // 16],
                        n_sg, n_sg, DO,
                        queue_num=gidx[0] % nq)
                    gidx[0] += 1
                    # sections whose last tile lands in this chunk: full
                    # selector + matmul chain + drain (msg read from the
                    # 1-2 covering chunk tiles, all still pool-live)
                    while si < len(secs) and secs[si][2] < c0 + cnt:
                        bb, t_lo, t_hi, kvar = secs[si]
                        si += 1
                        span = t_hi - t_lo + 1
                        sel = selp.tile([128, smax * 128], bf16, tag="sel")
                        s_view = sel[:, :span * 128].rearrange(
                            "p (t j) -> p t j", j=128)
                        d_ap = dst_sb[:, slot0 // 128 + t_lo:
                                      slot0 // 128 + t_hi + 1]
                        d_b = d_ap.unsqueeze(2).broadcast_to(
                            (128, span, 128))
                        i_b = iota4[:, kvar, :].unsqueeze(1).broadcast_to(
                            (128, span, 128))
                        nc.vector.tensor_tensor(s_view, d_b, i_b,
                                                mybir.AluOpType.is_equal)
                        ps = pspool.tile([128, DO], fp32, tag="ps")
                        for ti, t in enumerate(range(t_lo, t_hi + 1)):
                            nc.tensor.matmul(
                                ps[:, :], sel[:, ti * 128:(ti + 1) * 128],
                                msg_tiles[t // gchunk][:, t % gchunk, :],
                                start=(ti == 0), stop=(ti == span - 1))
                        # acc[:, bb, :] += dinv[bb] * ps
                        nc.vector.scalar_tensor_tensor(
                            acc[:, bb, :], ps[:, :], dinv[:, bb:bb + 1],
                            acc[:, bb, :],
                            mybir.AluOpType.mult, mybir.AluOpType.add)
                        if g == last_g[bb]:
                            t2 = epool.tile([128, DO], fp32, tag="t2")
                            nc.scalar.activation(
                                t2[:, :], acc[:, bb, :],
                                mybir.ActivationFunctionType.Relu)
                            rows = min(128, cfg.ns - bb * 128)
                            nc.sync.dma_start(
                                out[bb * 128:bb * 128 + rows, :],
                                t2[:rows, :])
                assert si == len(secs), (si, len(secs))
        nc.leave_named_scope("phC", _sidC, False)

    nc.compile()
    return nc


def kernel(x, edge_index, W, b):
    global LAST_RES
    cfg = Cfg(N_NODES, D_IN, D_OUT, N_CORES)
    in_maps, tot_slots = preprocess(x, edge_index, W, b, cfg)
    nc = build_program(cfg)
    res = run_bass_kernel_spmd(nc, in_maps, list(range(N_CORES)), trace=TRACE)
    LAST_RES = res
    outs = [r["out"][:cfg.ns] for r in res.results]
    return np.concatenate(outs, axis=0).astype(np.float32)


if __name__ == "__main__":
    cfg = Cfg(N_NODES, D_IN, D_OUT, N_CORES)
    print("cfg", cfg.nb, cfg.nsup, cfg.slice_blocks, cfg.group_rows)


# revision 14
# speedup vs baseline: 1.0302x; 1.0302x over previous
"""GCN (GCNConv + ReLU) message-passing kernel for 8 Trainium2 NeuronCores.

v3 strategy (dst-sharded graph parallelism, pipelined AllGather, chunked
gather stream):
  - Nodes sharded contiguously across 8 cores (12500 each, padded 12544).
  - The local node range is split into K=4 block-aligned slices. Phase A
    computes h'_c = dinv_c * (x_c @ W) (PE matmul, bf16 x staged on host,
    4 dst blocks per x DMA), writing each slice to its own internal DRAM
    buffer cc_in_j (fp16). As soon as slice j is written, AllGather_j
    broadcasts it: cc_out_j = concat over cores of slice j.
  - Edge srcs fall into the K "groups" (one per slice). Per core, edges
    are bucketed by (src group g, dst block b) and laid out in
    (g, super, b) order with sections packed tightly (padded only to the
    cross-core max per section and to 128 slots per (g, super) run).
  - Phase C runs group-major: gathers + segment-sum for group g only need
    AllGather_j=g, so groups 1..3 overlap with outstanding collectives.
  - Per-edge messages fetched with SWDGE dma_gather (int16 indices into
    cc_out_g), one gather call per 32-tile chunk (4096 edges) so the
    GpSimd descriptor-gen fixed cost is amortized and matmuls for chunk c
    can start while chunk c+1 is still gathering. Segment-sum via
    TensorE: one-hot selectors (fp16 is_equal of dst offsets vs
    per-section-shifted iota) contracted with fp16 message tiles in PSUM;
    per section, psum*dinv[b] is accumulated into an SBUF f32 accumulator
    preloaded with dinv^2*h + b (self loop).
  - Epilogue: ReLU(acc) -> out.

Host-side work is limited to integer index preprocessing (edge bucketing,
degree counts) and layout/dtype staging; all floating-point math runs on
device.
"""

import math
import sys

import numpy as np

sys.path.insert(0, "/opt/trn_rl_repo")

import ml_dtypes  # noqa: E402

import concourse.bass as bass  # noqa: E402
import concourse.bacc as bacc  # noqa: E402
import concourse.mybir as mybir  # noqa: E402
from concourse import tile  # noqa: E402
from concourse.bass_utils import run_bass_kernel_spmd  # noqa: E402

BF16 = ml_dtypes.bfloat16
FP16 = np.float16

# ----- problem constants (hardcoded; kernel.py must be self-contained) -----
N_NODES = 100000
D_IN = 256
D_OUT = 128
N_CORES = 8

# test-harness hooks (harness leaves these at defaults)
TRACE = False
LAST_RES = None


class Cfg:
    """Static, per-compile configuration (identical across cores)."""

    def __init__(self, n_nodes, d_in, d_out, n_cores, n_slices=4,
                 blocks_per_super=24):
        assert n_nodes % n_cores == 0
        self.n_nodes = n_nodes
        self.d_in = d_in
        self.d_out = d_out
        assert d_out == 128, "kernel assumes 128 output features"
        assert d_in % 128 == 0
        self.kchunks = d_in // 128
        self.n_cores = n_cores
        self.ns = n_nodes // n_cores          # nodes per core
        self.nb = math.ceil(self.ns / 128)    # dst blocks per core
        self.ns_pad = self.nb * 128
        self.bs = blocks_per_super
        self.nsup = math.ceil(self.nb / self.bs)
        # K block-aligned slices of the local node range
        K = min(n_slices, self.nb)
        # bathtub slice sizes: small first slice so AllGather0 fires early,
        # small last slice so the AG3-dependent tail is short
        # (each slice <=31 blocks for int16 gather indices)
        if self.nb == 98 and K == 4:
            self.slice_blocks = [8, 31, 31, 28]
        else:
            s0 = max(1, self.nb // (2 * K))
            base, rem = divmod(self.nb - s0, K - 1) if K > 1 else (0, 0)
            self.slice_blocks = [s0] + [base + (1 if j < rem else 0)
                                        for j in range(K - 1)]
        self.K = K
        self.slice_b0 = np.concatenate(
            [[0], np.cumsum(self.slice_blocks)]).astype(np.int64)
        self.slice_rows = [bl * 128 for bl in self.slice_blocks]
        # gather-group g corresponds to slice g of EVERY core:
        # cc_out_g rows = concat_c cc_in_g(core c), R_g = n_cores*rows_g
        self.group_rows = [n_cores * r for r in self.slice_rows]
        assert max(self.group_rows) <= 32767, "int16 gather index overflow"
        # filled by preprocessing:
        self.Lsec = None          # [K][nb] cross-core-max section sizes
        self.run_tiles = None     # [K][nsup] padded tiles per run
        self.run_slot0 = None     # [K][nsup] global slot offset
        self.sec_off = None       # {(g,b): offset of section inside its run}
        self.tot_slots = None

    def blocks_of_super(self, s):
        return range(s * self.bs, min((s + 1) * self.bs, self.nb))

    def slice_of_block(self, b):
        return int(np.searchsorted(self.slice_b0, b, side="right") - 1)


def preprocess(x, edge_index, W, b, cfg: Cfg):
    """Integer/layout-only host prep. Returns per-core input dicts."""
    n, ns, K = cfg.n_nodes, cfg.ns, cfg.K
    src = np.asarray(edge_index[0], dtype=np.int64)
    dst = np.asarray(edge_index[1], dtype=np.int64)
    x = np.asarray(x, dtype=np.float32)
    W = np.asarray(W, dtype=np.float32)
    b = np.asarray(b, dtype=np.float32)

    # map src global node -> (group, within-group row)
    s_core = src // ns
    s_loc = src % ns
    s_blk = s_loc // 128
    s_g = np.searchsorted(cfg.slice_b0, s_blk, side="right") - 1
    r0 = np.array([cfg.slice_b0[j] * 128 for j in range(K)])
    rows = np.array(cfg.slice_rows)
    s_idx = s_core * rows[s_g] + (s_loc - r0[s_g])  # within-group row

    core_of = dst // ns
    order = np.argsort(core_of, kind="stable")
    src_o, dst_o, sg_o, sidx_o = src[order], dst[order], s_g[order], s_idx[order]
    core_bounds = np.searchsorted(core_of[order], np.arange(cfg.n_cores + 1))

    percore = []
    counts = np.zeros((cfg.n_cores, K, cfg.nb), dtype=np.int64)
    for c in range(cfg.n_cores):
        lo, hi = core_bounds[c], core_bounds[c + 1]
        d_c = dst_o[lo:hi] - c * ns
        g_c = sg_o[lo:hi]
        i_c = sidx_o[lo:hi]
        blk = d_c // 128
        key = g_c * cfg.nb + blk
        o = np.argsort(key, kind="stable")
        d_c, g_c, i_c, key = d_c[o], g_c[o], i_c[o], key[o]
        counts[c] = np.bincount(key, minlength=K * cfg.nb).reshape(K, cfg.nb)
        deg = np.bincount(d_c, minlength=ns) + 1  # + self loop
        percore.append({"d": d_c, "g": g_c, "i": i_c, "key": key, "deg": deg})

    # cross-core-uniform (unrounded) section sizes
    Lsec = counts.max(axis=0)  # [K, nb]
    cfg.Lsec = Lsec

    # run layout: for g, for super s: sections (g, b in super) tight,
    # run padded to a multiple of 128 slots
    run_tiles = np.zeros((K, cfg.nsup), dtype=np.int64)
    run_slot0 = np.zeros((K, cfg.nsup), dtype=np.int64)
    sec_off = {}
    off = 0
    for g in range(K):
        for s in range(cfg.nsup):
            run_slot0[g, s] = off
            roff = 0
            for bb in cfg.blocks_of_super(s):
                sec_off[(g, bb)] = roff
                roff += int(Lsec[g, bb])
            rt = (roff + 127) // 128
            run_tiles[g, s] = rt
            off += rt * 128
    tot_slots = off
    cfg.run_tiles = run_tiles
    cfg.run_slot0 = run_slot0
    cfg.sec_off = sec_off
    cfg.tot_slots = tot_slots

    # verify <=4 sections per tile and no iota-offset collision within a tile
    for g in range(K):
        for s in range(cfg.nsup):
            bounds = []
            for bb in cfg.blocks_of_super(s):
                if Lsec[g, bb] > 0:
                    o0 = sec_off[(g, bb)]
                    bounds.append((o0, o0 + int(Lsec[g, bb]), bb - s * cfg.bs))
            for t in range(int(run_tiles[g, s])):
                lo, hi = t * 128, (t + 1) * 128
                ords = [o % 4 for (a, bnd, o) in bounds if a < hi and bnd > lo]
                assert len(ords) == len(set(ords)), "iota offset collision"

    in_maps = []
    for c in range(cfg.n_cores):
        pc = percore[c]
        idx_all = np.zeros(tot_slots, dtype=np.int16)
        dst_all = np.full(tot_slots, -1.0, dtype=np.float32)
        cnt = np.bincount(pc["key"], minlength=K * cfg.nb).reshape(K, cfg.nb)
        flat = 0
        for g in range(K):
            for bb in range(cfg.nb):
                m = int(cnt[g, bb])
                if m:
                    sl = slice(flat, flat + m)
                    s = bb // cfg.bs
                    o0 = int(cfg.run_slot0[g, s]) + cfg.sec_off[(g, bb)]
                    idx_all[o0:o0 + m] = pc["i"][sl].astype(np.int16)
                    oshift = 128.0 * ((bb - s * cfg.bs) % 4)
                    dst_all[o0:o0 + m] = (
                        pc["d"][sl] - bb * 128).astype(np.float32) + oshift
                flat += m
        # wrap idx into 16 partitions, replicated to 128
        idx_w16 = idx_all.reshape(-1, 16).T.copy()          # [16, tot/16]
        idx_w = np.tile(idx_w16, (8, 1))                     # [128, tot/16]
        dst_w = np.ascontiguousarray(
            dst_all.reshape(-1, 128).T).astype(FP16)         # [128, tot/128]

        deg_pad = np.ones(cfg.ns_pad, dtype=np.int32)
        deg_pad[:ns] = pc["deg"].astype(np.int32)
        deg_w = np.ascontiguousarray(deg_pad.reshape(cfg.nb, 128).T)  # [128, nb]

        xs = x[c * ns:(c + 1) * ns]
        xT = np.zeros((cfg.d_in, cfg.ns_pad), dtype=BF16)
        xT[:, :ns] = xs.T.astype(BF16)

        in_maps.append({
            "xT": xT,
            "Wm": W.copy(),
            "bias": b.reshape(1, -1).copy(),
            "deg": deg_w,
            "idx": idx_w,
            "dstloc": dst_w,
        })
    return in_maps, tot_slots


def build_program(cfg: Cfg, mock_cc=False, gchunk=8, dds=32768, nq=4):
    """Builds the SPMD bass program (same NEFF on every core)."""
    fp32 = mybir.dt.float32
    bf16 = mybir.dt.bfloat16
    fp16 = mybir.dt.float16
    K = cfg.K
    DO = cfg.d_out
    tot_slots = cfg.tot_slots
    Lsec = cfg.Lsec

    nc = bacc.Bacc("TRN2", target_bir_lowering=False,
                   num_devices=cfg.n_cores, debug=False,
                   dynamic_dma_scratch_size=dds,
                   num_swdge_queues=nq)

    xT = nc.dram_tensor("xT", [cfg.d_in, cfg.ns_pad], bf16, kind="ExternalInput")
    Wm = nc.dram_tensor("Wm", [cfg.d_in, DO], fp32, kind="ExternalInput")
    bias = nc.dram_tensor("bias", [1, DO], fp32, kind="ExternalInput")
    deg = nc.dram_tensor("deg", [128, cfg.nb], mybir.dt.int32, kind="ExternalInput")
    idx = nc.dram_tensor("idx", [128, tot_slots // 16], mybir.dt.int16,
                         kind="ExternalInput")
    dstloc = nc.dram_tensor("dstloc", [128, tot_slots // 128], fp16,
                            kind="ExternalInput")
    out = nc.dram_tensor("out", [cfg.ns, DO], fp32, kind="ExternalOutput")

    cc_in = [nc.dram_tensor(f"cc_in{j}", [cfg.slice_rows[j], DO], bf16)
             for j in range(K)]
    cc_out = [nc.dram_tensor(f"cc_out{j}", [cfg.group_rows[j], DO], bf16,
                             addr_space="Shared")
              for j in range(K)]

    # max section span in tiles (for sel tile sizing)
    max_span = 1
    for g in range(K):
        for bb in range(cfg.nb):
            ln = int(Lsec[g, bb])
            if ln == 0:
                continue
            o0 = cfg.sec_off[(g, bb)]
            max_span = max(max_span, (o0 + ln - 1) // 128 - o0 // 128 + 1)
    smax = max_span
    max_run_tiles = int(cfg.run_tiles.max())

    from contextlib import ExitStack
    with tile.TileContext(nc) as tc, ExitStack() as ctx:
        const = ctx.enter_context(tc.tile_pool(name="const", bufs=1))
        xpool = ctx.enter_context(tc.tile_pool(name="x", bufs=3))
        hpsum = ctx.enter_context(tc.tile_pool(name="hpsum", bufs=4, space="PSUM"))
        htmp = ctx.enter_context(tc.tile_pool(name="htmp", bufs=4))
        msgp = ctx.enter_context(tc.tile_pool(name="msg", bufs=8))
        idxp = ctx.enter_context(tc.tile_pool(name="idx", bufs=3))
        selp = ctx.enter_context(tc.tile_pool(name="sel", bufs=4))
        pspool = ctx.enter_context(tc.tile_pool(name="ps", bufs=4, space="PSUM"))
        epool = ctx.enter_context(tc.tile_pool(name="ep", bufs=4))

        # ---------------- phase 0: constants ----------------
        W_f = const.tile([128, cfg.kchunks, DO], fp32)
        for k in range(cfg.kchunks):
            nc.sync.dma_start(W_f[:, k, :], Wm[k * 128:(k + 1) * 128, :])
        W_b = const.tile([128, cfg.kchunks, DO], bf16)
        nc.vector.tensor_copy(W_b[:, :, :], W_f[:, :, :])

        deg_i = const.tile([128, cfg.nb], mybir.dt.int32)
        nc.sync.dma_start(deg_i[:, :], deg[:, :])
        deg_f = const.tile([128, cfg.nb], fp32)
        nc.vector.tensor_copy(deg_f[:, :], deg_i[:, :])
        deg_sq = const.tile([128, cfg.nb], fp32)
        nc.scalar.activation(deg_sq[:, :], deg_f[:, :],
                             mybir.ActivationFunctionType.Sqrt)
        dinv = const.tile([128, cfg.nb], fp32)
        nc.vector.reciprocal(dinv[:, :], deg_sq[:, :])
        dinv2 = const.tile([128, cfg.nb], fp32)
        nc.vector.tensor_tensor(dinv2[:, :], dinv[:, :], dinv[:, :],
                                mybir.AluOpType.mult)

        b_row = const.tile([1, DO], fp32)
        nc.sync.dma_start(b_row[:, :], bias[:, :])
        ones_row = const.tile([1, 128], fp32)
        nc.vector.memset(ones_row[:, :], 1.0)
        bt_ps = pspool.tile([128, DO], fp32, tag="ps")
        nc.tensor.matmul(bt_ps[:, :], ones_row[:, :], b_row[:, :],
                         start=True, stop=True)
        b_tile = const.tile([128, DO], fp32)
        nc.vector.tensor_copy(b_tile[:, :], bt_ps[:, :])

        # 4 iota variants (j + 128*k), fp16
        iota4 = const.tile([128, 4, 128], fp16)
        nc.gpsimd.iota(iota4[:, 0, :], [[1, 128]], channel_multiplier=0,
                       allow_small_or_imprecise_dtypes=True)
        for k in range(1, 4):
            nc.vector.tensor_scalar(iota4[:, k, :], iota4[:, 0, :],
                                    float(128 * k), None,
                                    mybir.AluOpType.add)

        dst_sb = const.tile([128, tot_slots // 128], fp16)

        # SBUF f32 accumulator; phase A preloads dinv^2*h + b (self loop)
        acc = const.tile([128, cfg.nb, DO], fp32)

        # ---------------- phase A + pipelined AllGathers ----------------
        _sidA, _ = nc.enter_named_scope("phA", False)
        for j in range(K):
            b_lo, b_hi = int(cfg.slice_b0[j]), int(cfg.slice_b0[j + 1])
            for k0 in range(b_lo, b_hi, 4):
                kw = min(4, b_hi - k0)
                xb = xpool.tile([128, cfg.kchunks, 4 * 128], bf16, tag="xb")
                nc.sync.dma_start(
                    xb[:, :, :kw * 128],
                    xT[:, k0 * 128:(k0 + kw) * 128].rearrange(
                        "(c p) n -> p c n", p=128))
                for k in range(k0, k0 + kw):
                    co = (k - k0) * 128
                    ph = hpsum.tile([128, DO], fp32, tag="hps")
                    for kb in range(cfg.kchunks):
                        nc.tensor.matmul(ph[:, :], xb[:, kb, co:co + 128],
                                         W_b[:, kb, :], start=(kb == 0),
                                         stop=(kb == cfg.kchunks - 1))
                    # self-loop + bias term preloaded into the accumulator
                    nc.vector.scalar_tensor_tensor(
                        acc[:, k, :], ph[:, :], dinv2[:, k:k + 1], b_tile[:, :],
                        mybir.AluOpType.mult, mybir.AluOpType.add)
                    # h' (fp16) -> cc_in[j]; scale on the (idle) scalar
                    # engine so phC selector builds don't stall phase A
                    hb = htmp.tile([128, DO], bf16, tag="hb")
                    nc.scalar.activation(hb[:, :], ph[:, :],
                                         mybir.ActivationFunctionType.Copy,
                                         scale=dinv[:, k:k + 1])
                    kr = k * 128 - b_lo * 128
                    nc.sync.dma_start(cc_in[j][kr:kr + 128, :], hb[:, :])
        nc.leave_named_scope("phA", _sidA, False)

        # phC-only constant, loaded behind phase A's x stream
        nc.sync.dma_start(dst_sb[:, :], dstloc[:, :])

        # AllGather triggers, emitted back-to-back so later groups'
        # collectives overlap with earlier groups' gathers/compute.
        _sidB, _ = nc.enter_named_scope("allgather", False)
        for j in range(K):
            if cfg.n_cores > 1 and not mock_cc:
                nc.gpsimd.collective_compute(
                    "AllGather",
                    mybir.AluOpType.bypass,
                    replica_groups=[list(range(cfg.n_cores))],
                    ins=[cc_in[j][:, :]],
                    outs=[cc_out[j][:, :]],
                )
            else:
                for cpy in range(cfg.n_cores):
                    r = cfg.slice_rows[j]
                    nc.sync.dma_start(cc_out[j][cpy * r:(cpy + 1) * r, :],
                                      cc_in[j][:, :])
        nc.leave_named_scope("allgather", _sidB, False)

        # ---------------- phase C: gather + segment-sum + epilogue -------
        _sidC, _ = nc.enter_named_scope("phC", False)
        gidx = [0]
        last_g = {}
        for bb in range(cfg.nb):
            lg = -1
            for g in range(K):
                if int(Lsec[g, bb]) > 0:
                    lg = g
            last_g[bb] = lg
        for bb in range(cfg.nb):
            if last_g[bb] < 0:
                t2 = epool.tile([128, DO], fp32, tag="t2")
                nc.scalar.activation(t2[:, :], acc[:, bb, :],
                                     mybir.ActivationFunctionType.Relu)
                rows = min(128, cfg.ns - bb * 128)
                nc.sync.dma_start(out[bb * 128:bb * 128 + rows, :],
                                  t2[:rows, :])

        for g in range(K):
            for s in range(cfg.nsup):
                ntiles = int(cfg.run_tiles[g, s])
                if ntiles == 0:
                    continue
                slot0 = int(cfg.run_slot0[g, s])
                idx_t = idxp.tile([128, max_run_tiles * 8], mybir.dt.int16,
                                  tag="idx")
                nc.sync.dma_start(idx_t[:, :ntiles * 8],
                                  idx[:, slot0 // 16:
                                      (slot0 + ntiles * 128) // 16])
                # section list for this run: (bb, t_lo, t_hi, kvar)
                secs = []
                for bb in cfg.blocks_of_super(s):
                    ln = int(Lsec[g, bb])
                    if ln == 0:
                        continue
                    o0 = cfg.sec_off[(g, bb)]
                    secs.append((bb, o0 // 128, (o0 + ln - 1) // 128,
                                 (bb - s * cfg.bs) % 4))
                nchunks = (ntiles + gchunk - 1) // gchunk
                msg_tiles = [None] * nchunks
                si = 0
                for c in range(nchunks):
                    c0 = c * gchunk
                    cnt = min(gchunk, ntiles - c0)
                    mt = msgp.tile([128, gchunk, 128], bf16, name="msg_ch",
                                   tag="msg")
                    msg_tiles[c] = mt
                    n_sg = cnt * 128
                    nc.gpsimd.dma_gather(
                        mt[:, :cnt, :],
                        cc_out[g][:, :],
                        idx_t[:, c0 * 8:c0 * 8 + n_sg // 16],
                        n_sg, n_sg, DO,
                        queue_num=gidx[0] % nq)
                    gidx[0] += 1
                    # sections whose last tile lands in this chunk: full
                    # selector + matmul chain + drain (msg read from the
                    # 1-2 covering chunk tiles, all still pool-live)
                    while si < len(secs) and secs[si][2] < c0 + cnt:
                        bb, t_lo, t_hi, kvar = secs[si]
                        si += 1
                        span = t_hi - t_lo + 1
                        sel = selp.tile([128, smax * 128], bf16, tag="sel")
                        s_view = sel[:, :span * 128].rearrange(
                            "p (t j) -> p t j", j=128)
                        d_ap = dst_sb[:, slot0 // 128 + t_lo:
                                      slot0 // 128 + t_hi + 1]
                        d_b = d_ap.unsqueeze(2).broadcast_to(
                            (128, span, 128))
                        i_b = iota4[:, kvar, :].unsqueeze(1).broadcast_to(
                            (128, span, 128))
                        nc.vector.tensor_tensor(s_view, d_b, i_b,
                                                mybir.AluOpType.is_equal)
                        ps = pspool.tile([128, DO], fp32, tag="ps")
                        for ti, t in enumerate(range(t_lo, t_hi + 1)):
                            nc.tensor.matmul(
                                ps[:, :], sel[:, ti * 128:(ti + 1) * 128],
                                msg_tiles[t // gchunk][:, t % gchunk, :],
                                start=(ti == 0), stop=(ti == span - 1))
                        # acc[:, bb, :] += dinv[bb] * ps
                        nc.vector.scalar_tensor_tensor(
                            acc[:, bb, :], ps[:, :], dinv[:, bb:bb + 1],
                            acc[:, bb, :],
                            mybir.AluOpType.mult, mybir.AluOpType.add)
                        if g == last_g[bb]:
                            t2 = epool.tile([128, DO], fp32, tag="t2")
                            nc.scalar.activation(
                                t2[:, :], acc[:, bb, :],
                                mybir.ActivationFunctionType.Relu)
                            rows = min(128, cfg.ns - bb * 128)
                            nc.sync.dma_start(
                                out[bb * 128:bb * 128 + rows, :],
                                t2[:rows, :])
                assert si == len(secs), (si, len(secs))
        nc.leave_named_scope("phC", _sidC, False)

    nc.compile()
    return nc


def kernel(x, edge_index, W, b):
    global LAST_RES
    cfg = Cfg(N_NODES, D_IN, D_OUT, N_CORES)
    in_maps, tot_slots = preprocess(x, edge_index, W, b, cfg)
    nc = build_program(cfg)
    res = run_bass_kernel_spmd(nc, in_maps, list(range(N_CORES)), trace=TRACE)
    LAST_RES = res
    outs = [r["out"][:cfg.ns] for r in res.results]
    return np.concatenate(outs, axis=0).astype(np.float32)


if __name__ == "__main__":
    cfg = Cfg(N_NODES, D_IN, D_OUT, N_CORES)
    print("cfg", cfg.nb, cfg.nsup, cfg.slice_blocks, cfg.group_rows)


# revision 15
# speedup vs baseline: 1.0302x; 1.0001x over previous
"""GCN (GCNConv + ReLU) message-passing kernel for 8 Trainium2 NeuronCores.

v3 strategy (dst-sharded graph parallelism, pipelined AllGather, chunked
gather stream):
  - Nodes sharded contiguously across 8 cores (12500 each, padded 12544).
  - The local node range is split into K=4 block-aligned slices. Phase A
    computes h'_c = dinv_c * (x_c @ W) (PE matmul, bf16 x staged on host,
    4 dst blocks per x DMA), writing each slice to its own internal DRAM
    buffer cc_in_j (fp16). As soon as slice j is written, AllGather_j
    broadcasts it: cc_out_j = concat over cores of slice j.
  - Edge srcs fall into the K "groups" (one per slice). Per core, edges
    are bucketed by (src group g, dst block b) and laid out in
    (g, super, b) order with sections packed tightly (padded only to the
    cross-core max per section and to 128 slots per (g, super) run).
  - Phase C runs group-major: gathers + segment-sum for group g only need
    AllGather_j=g, so groups 1..3 overlap with outstanding collectives.
  - Per-edge messages fetched with SWDGE dma_gather (int16 indices into
    cc_out_g), one gather call per 32-tile chunk (4096 edges) so the
    GpSimd descriptor-gen fixed cost is amortized and matmuls for chunk c
    can start while chunk c+1 is still gathering. Segment-sum via
    TensorE: one-hot selectors (fp16 is_equal of dst offsets vs
    per-section-shifted iota) contracted with fp16 message tiles in PSUM;
    per section, psum*dinv[b] is accumulated into an SBUF f32 accumulator
    preloaded with dinv^2*h + b (self loop).
  - Epilogue: ReLU(acc) -> out.

Host-side work is limited to integer index preprocessing (edge bucketing,
degree counts) and layout/dtype staging; all floating-point math runs on
device.
"""

import math
import sys

import numpy as np

sys.path.insert(0, "/opt/trn_rl_repo")

import ml_dtypes  # noqa: E402

import concourse.bass as bass  # noqa: E402
import concourse.bacc as bacc  # noqa: E402
import concourse.mybir as mybir  # noqa: E402
from concourse import tile  # noqa: E402
from concourse.bass_utils import run_bass_kernel_spmd  # noqa: E402

BF16 = ml_dtypes.bfloat16
FP16 = np.float16

# ----- problem constants (hardcoded; kernel.py must be self-contained) -----
N_NODES = 100000
D_IN = 256
D_OUT = 128
N_CORES = 8

# test-harness hooks (harness leaves these at defaults)
TRACE = False
LAST_RES = None


class Cfg:
    """Static, per-compile configuration (identical across cores)."""

    def __init__(self, n_nodes, d_in, d_out, n_cores, n_slices=4,
                 blocks_per_super=24):
        assert n_nodes % n_cores == 0
        self.n_nodes = n_nodes
        self.d_in = d_in
        self.d_out = d_out
        assert d_out == 128, "kernel assumes 128 output features"
        assert d_in % 128 == 0
        self.kchunks = d_in // 128
        self.n_cores = n_cores
        self.ns = n_nodes // n_cores          # nodes per core
        self.nb = math.ceil(self.ns / 128)    # dst blocks per core
        self.ns_pad = self.nb * 128
        self.bs = blocks_per_super
        self.nsup = math.ceil(self.nb / self.bs)
        # K block-aligned slices of the local node range
        K = min(n_slices, self.nb)
        # bathtub slice sizes: small first slice so AllGather0 fires early,
        # small last slice so the AG3-dependent tail is short
        # (each slice <=31 blocks for int16 gather indices)
        if self.nb == 98 and K == 4:
            self.slice_blocks = [12, 29, 31, 26]
        else:
            s0 = max(1, self.nb // (2 * K))
            base, rem = divmod(self.nb - s0, K - 1) if K > 1 else (0, 0)
            self.slice_blocks = [s0] + [base + (1 if j < rem else 0)
                                        for j in range(K - 1)]
        self.K = K
        self.slice_b0 = np.concatenate(
            [[0], np.cumsum(self.slice_blocks)]).astype(np.int64)
        self.slice_rows = [bl * 128 for bl in self.slice_blocks]
        # gather-group g corresponds to slice g of EVERY core:
        # cc_out_g rows = concat_c cc_in_g(core c), R_g = n_cores*rows_g
        self.group_rows = [n_cores * r for r in self.slice_rows]
        assert max(self.group_rows) <= 32767, "int16 gather index overflow"
        # filled by preprocessing:
        self.Lsec = None          # [K][nb] cross-core-max section sizes
        self.run_tiles = None     # [K][nsup] padded tiles per run
        self.run_slot0 = None     # [K][nsup] global slot offset
        self.sec_off = None       # {(g,b): offset of section inside its run}
        self.tot_slots = None

    def blocks_of_super(self, s):
        return range(s * self.bs, min((s + 1) * self.bs, self.nb))

    def slice_of_block(self, b):
        return int(np.searchsorted(self.slice_b0, b, side="right") - 1)


def preprocess(x, edge_index, W, b, cfg: Cfg):
    """Integer/layout-only host prep. Returns per-core input dicts."""
    n, ns, K = cfg.n_nodes, cfg.ns, cfg.K
    src = np.asarray(edge_index[0], dtype=np.int64)
    dst = np.asarray(edge_index[1], dtype=np.int64)
    x = np.asarray(x, dtype=np.float32)
    W = np.asarray(W, dtype=np.float32)
    b = np.asarray(b, dtype=np.float32)

    # map src global node -> (group, within-group row)
    s_core = src // ns
    s_loc = src % ns
    s_blk = s_loc // 128
    s_g = np.searchsorted(cfg.slice_b0, s_blk, side="right") - 1
    r0 = np.array([cfg.slice_b0[j] * 128 for j in range(K)])
    rows = np.array(cfg.slice_rows)
    s_idx = s_core * rows[s_g] + (s_loc - r0[s_g])  # within-group row

    core_of = dst // ns
    order = np.argsort(core_of, kind="stable")
    src_o, dst_o, sg_o, sidx_o = src[order], dst[order], s_g[order], s_idx[order]
    core_bounds = np.searchsorted(core_of[order], np.arange(cfg.n_cores + 1))

    percore = []
    counts = np.zeros((cfg.n_cores, K, cfg.nb), dtype=np.int64)
    for c in range(cfg.n_cores):
        lo, hi = core_bounds[c], core_bounds[c + 1]
        d_c = dst_o[lo:hi] - c * ns
        g_c = sg_o[lo:hi]
        i_c = sidx_o[lo:hi]
        blk = d_c // 128
        key = g_c * cfg.nb + blk
        o = np.argsort(key, kind="stable")
        d_c, g_c, i_c, key = d_c[o], g_c[o], i_c[o], key[o]
        counts[c] = np.bincount(key, minlength=K * cfg.nb).reshape(K, cfg.nb)
        deg = np.bincount(d_c, minlength=ns) + 1  # + self loop
        percore.append({"d": d_c, "g": g_c, "i": i_c, "key": key, "deg": deg})

    # cross-core-uniform (unrounded) section sizes
    Lsec = counts.max(axis=0)  # [K, nb]
    cfg.Lsec = Lsec

    # run layout: for g, for super s: sections (g, b in super) tight,
    # run padded to a multiple of 128 slots
    run_tiles = np.zeros((K, cfg.nsup), dtype=np.int64)
    run_slot0 = np.zeros((K, cfg.nsup), dtype=np.int64)
    sec_off = {}
    off = 0
    for g in range(K):
        for s in range(cfg.nsup):
            run_slot0[g, s] = off
            roff = 0
            for bb in cfg.blocks_of_super(s):
                sec_off[(g, bb)] = roff
                roff += int(Lsec[g, bb])
            rt = (roff + 127) // 128
            run_tiles[g, s] = rt
            off += rt * 128
    tot_slots = off
    cfg.run_tiles = run_tiles
    cfg.run_slot0 = run_slot0
    cfg.sec_off = sec_off
    cfg.tot_slots = tot_slots

    # verify <=4 sections per tile and no iota-offset collision within a tile
    for g in range(K):
        for s in range(cfg.nsup):
            bounds = []
            for bb in cfg.blocks_of_super(s):
                if Lsec[g, bb] > 0:
                    o0 = sec_off[(g, bb)]
                    bounds.append((o0, o0 + int(Lsec[g, bb]), bb - s * cfg.bs))
            for t in range(int(run_tiles[g, s])):
                lo, hi = t * 128, (t + 1) * 128
                ords = [o % 4 for (a, bnd, o) in bounds if a < hi and bnd > lo]
                assert len(ords) == len(set(ords)), "iota offset collision"

    in_maps = []
    for c in range(cfg.n_cores):
        pc = percore[c]
        idx_all = np.zeros(tot_slots, dtype=np.int16)
        dst_all = np.full(tot_slots, -1.0, dtype=np.float32)
        cnt = np.bincount(pc["key"], minlength=K * cfg.nb).reshape(K, cfg.nb)
        flat = 0
        for g in range(K):
            for bb in range(cfg.nb):
                m = int(cnt[g, bb])
                if m:
                    sl = slice(flat, flat + m)
                    s = bb // cfg.bs
                    o0 = int(cfg.run_slot0[g, s]) + cfg.sec_off[(g, bb)]
                    idx_all[o0:o0 + m] = pc["i"][sl].astype(np.int16)
                    oshift = 128.0 * ((bb - s * cfg.bs) % 4)
                    dst_all[o0:o0 + m] = (
                        pc["d"][sl] - bb * 128).astype(np.float32) + oshift
                flat += m
        # wrap idx into 16 partitions, replicated to 128
        idx_w16 = idx_all.reshape(-1, 16).T.copy()          # [16, tot/16]
        idx_w = np.tile(idx_w16, (8, 1))                     # [128, tot/16]
        dst_w = np.ascontiguousarray(
            dst_all.reshape(-1, 128).T).astype(FP16)         # [128, tot/128]

        deg_pad = np.ones(cfg.ns_pad, dtype=np.int32)
        deg_pad[:ns] = pc["deg"].astype(np.int32)
        deg_w = np.ascontiguousarray(deg_pad.reshape(cfg.nb, 128).T)  # [128, nb]

        xs = x[c * ns:(c + 1) * ns]
        xT = np.zeros((cfg.d_in, cfg.ns_pad), dtype=BF16)
        xT[:, :ns] = xs.T.astype(BF16)

        in_maps.append({
            "xT": xT,
            "Wm": W.copy(),
            "bias": b.reshape(1, -1).copy(),
            "deg": deg_w,
            "idx": idx_w,
            "dstloc": dst_w,
        })
    return in_maps, tot_slots


def build_program(cfg: Cfg, mock_cc=False, gchunk=8, dds=32768, nq=4):
    """Builds the SPMD bass program (same NEFF on every core)."""
    fp32 = mybir.dt.float32
    bf16 = mybir.dt.bfloat16
    fp16 = mybir.dt.float16
    K = cfg.K
    DO = cfg.d_out
    tot_slots = cfg.tot_slots
    Lsec = cfg.Lsec

    nc = bacc.Bacc("TRN2", target_bir_lowering=False,
                   num_devices=cfg.n_cores, debug=False,
                   dynamic_dma_scratch_size=dds,
                   num_swdge_queues=nq)

    xT = nc.dram_tensor("xT", [cfg.d_in, cfg.ns_pad], bf16, kind="ExternalInput")
    Wm = nc.dram_tensor("Wm", [cfg.d_in, DO], fp32, kind="ExternalInput")
    bias = nc.dram_tensor("bias", [1, DO], fp32, kind="ExternalInput")
    deg = nc.dram_tensor("deg", [128, cfg.nb], mybir.dt.int32, kind="ExternalInput")
    idx = nc.dram_tensor("idx", [128, tot_slots // 16], mybir.dt.int16,
                         kind="ExternalInput")
    dstloc = nc.dram_tensor("dstloc", [128, tot_slots // 128], fp16,
                            kind="ExternalInput")
    out = nc.dram_tensor("out", [cfg.ns, DO], fp32, kind="ExternalOutput")

    cc_in = [nc.dram_tensor(f"cc_in{j}", [cfg.slice_rows[j], DO], bf16)
             for j in range(K)]
    cc_out = [nc.dram_tensor(f"cc_out{j}", [cfg.group_rows[j], DO], bf16,
                             addr_space="Shared")
              for j in range(K)]

    # max section span in tiles (for sel tile sizing)
    max_span = 1
    for g in range(K):
        for bb in range(cfg.nb):
            ln = int(Lsec[g, bb])
            if ln == 0:
                continue
            o0 = cfg.sec_off[(g, bb)]
            max_span = max(max_span, (o0 + ln - 1) // 128 - o0 // 128 + 1)
    smax = max_span
    max_run_tiles = int(cfg.run_tiles.max())

    from contextlib import ExitStack
    with tile.TileContext(nc) as tc, ExitStack() as ctx:
        const = ctx.enter_context(tc.tile_pool(name="const", bufs=1))
        xpool = ctx.enter_context(tc.tile_pool(name="x", bufs=3))
        hpsum = ctx.enter_context(tc.tile_pool(name="hpsum", bufs=4, space="PSUM"))
        htmp = ctx.enter_context(tc.tile_pool(name="htmp", bufs=4))
        msgp = ctx.enter_context(tc.tile_pool(name="msg", bufs=12))
        idxp = ctx.enter_context(tc.tile_pool(name="idx", bufs=4))
        selp = ctx.enter_context(tc.tile_pool(name="sel", bufs=6))
        pspool = ctx.enter_context(tc.tile_pool(name="ps", bufs=4, space="PSUM"))
        epool = ctx.enter_context(tc.tile_pool(name="ep", bufs=4))

        # ---------------- phase 0: constants ----------------
        W_f = const.tile([128, cfg.kchunks, DO], fp32)
        for k in range(cfg.kchunks):
            nc.sync.dma_start(W_f[:, k, :], Wm[k * 128:(k + 1) * 128, :])
        W_b = const.tile([128, cfg.kchunks, DO], bf16)
        nc.vector.tensor_copy(W_b[:, :, :], W_f[:, :, :])

        deg_i = const.tile([128, cfg.nb], mybir.dt.int32)
        nc.sync.dma_start(deg_i[:, :], deg[:, :])
        deg_f = const.tile([128, cfg.nb], fp32)
        nc.vector.tensor_copy(deg_f[:, :], deg_i[:, :])
        deg_sq = const.tile([128, cfg.nb], fp32)
        nc.scalar.activation(deg_sq[:, :], deg_f[:, :],
                             mybir.ActivationFunctionType.Sqrt)
        dinv = const.tile([128, cfg.nb], fp32)
        nc.vector.reciprocal(dinv[:, :], deg_sq[:, :])
        dinv2 = const.tile([128, cfg.nb], fp32)
        nc.vector.tensor_tensor(dinv2[:, :], dinv[:, :], dinv[:, :],
                                mybir.AluOpType.mult)

        b_row = const.tile([1, DO], fp32)
        nc.sync.dma_start(b_row[:, :], bias[:, :])
        ones_row = const.tile([1, 128], fp32)
        nc.vector.memset(ones_row[:, :], 1.0)
        bt_ps = pspool.tile([128, DO], fp32, tag="ps")
        nc.tensor.matmul(bt_ps[:, :], ones_row[:, :], b_row[:, :],
                         start=True, stop=True)
        b_tile = const.tile([128, DO], fp32)
        nc.vector.tensor_copy(b_tile[:, :], bt_ps[:, :])

        # 4 iota variants (j + 128*k), fp16
        iota4 = const.tile([128, 4, 128], fp16)
        nc.gpsimd.iota(iota4[:, 0, :], [[1, 128]], channel_multiplier=0,
                       allow_small_or_imprecise_dtypes=True)
        for k in range(1, 4):
            nc.vector.tensor_scalar(iota4[:, k, :], iota4[:, 0, :],
                                    float(128 * k), None,
                                    mybir.AluOpType.add)

        dst_sb = const.tile([128, tot_slots // 128], fp16)

        # SBUF f32 accumulator; phase A preloads dinv^2*h + b (self loop)
        acc = const.tile([128, cfg.nb, DO], fp32)

        # ---------------- phase A + pipelined AllGathers ----------------
        _sidA, _ = nc.enter_named_scope("phA", False)
        for j in range(K):
            b_lo, b_hi = int(cfg.slice_b0[j]), int(cfg.slice_b0[j + 1])
            for k0 in range(b_lo, b_hi, 4):
                kw = min(4, b_hi - k0)
                xb = xpool.tile([128, cfg.kchunks, 4 * 128], bf16, tag="xb")
                nc.sync.dma_start(
                    xb[:, :, :kw * 128],
                    xT[:, k0 * 128:(k0 + kw) * 128].rearrange(
                        "(c p) n -> p c n", p=128))
                for k in range(k0, k0 + kw):
                    co = (k - k0) * 128
                    ph = hpsum.tile([128, DO], fp32, tag="hps")
                    for kb in range(cfg.kchunks):
                        nc.tensor.matmul(ph[:, :], xb[:, kb, co:co + 128],
                                         W_b[:, kb, :], start=(kb == 0),
                                         stop=(kb == cfg.kchunks - 1))
                    # self-loop + bias term preloaded into the accumulator
                    nc.vector.scalar_tensor_tensor(
                        acc[:, k, :], ph[:, :], dinv2[:, k:k + 1], b_tile[:, :],
                        mybir.AluOpType.mult, mybir.AluOpType.add)
                    # h' (fp16) -> cc_in[j]; scale on the (idle) scalar
                    # engine so phC selector builds don't stall phase A
                    hb = htmp.tile([128, DO], bf16, tag="hb")
                    nc.scalar.activation(hb[:, :], ph[:, :],
                                         mybir.ActivationFunctionType.Copy,
                                         scale=dinv[:, k:k + 1])
                    kr = k * 128 - b_lo * 128
                    nc.sync.dma_start(cc_in[j][kr:kr + 128, :], hb[:, :])
        nc.leave_named_scope("phA", _sidA, False)

        # phC-only constant, loaded behind phase A's x stream
        nc.sync.dma_start(dst_sb[:, :], dstloc[:, :])

        # AllGather triggers, emitted back-to-back so later groups'
        # collectives overlap with earlier groups' gathers/compute.
        _sidB, _ = nc.enter_named_scope("allgather", False)
        for j in range(K):
            if cfg.n_cores > 1 and not mock_cc:
                nc.gpsimd.collective_compute(
                    "AllGather",
                    mybir.AluOpType.bypass,
                    replica_groups=[list(range(cfg.n_cores))],
                    ins=[cc_in[j][:, :]],
                    outs=[cc_out[j][:, :]],
                )
            else:
                for cpy in range(cfg.n_cores):
                    r = cfg.slice_rows[j]
                    nc.sync.dma_start(cc_out[j][cpy * r:(cpy + 1) * r, :],
                                      cc_in[j][:, :])
        nc.leave_named_scope("allgather", _sidB, False)

        # ---------------- phase C: gather + segment-sum + epilogue -------
        _sidC, _ = nc.enter_named_scope("phC", False)
        gidx = [0]
        last_g = {}
        for bb in range(cfg.nb):
            lg = -1
            for g in range(K):
                if int(Lsec[g, bb]) > 0:
                    lg = g
            last_g[bb] = lg
        for bb in range(cfg.nb):
            if last_g[bb] < 0:
                t2 = epool.tile([128, DO], fp32, tag="t2")
                nc.scalar.activation(t2[:, :], acc[:, bb, :],
                                     mybir.ActivationFunctionType.Relu)
                rows = min(128, cfg.ns - bb * 128)
                nc.sync.dma_start(out[bb * 128:bb * 128 + rows, :],
                                  t2[:rows, :])

        for g in range(K):
            for s in range(cfg.nsup):
                ntiles = int(cfg.run_tiles[g, s])
                if ntiles == 0:
                    continue
                slot0 = int(cfg.run_slot0[g, s])
                idx_t = idxp.tile([128, max_run_tiles * 8], mybir.dt.int16,
                                  tag="idx")
                nc.sync.dma_start(idx_t[:, :ntiles * 8],
                                  idx[:, slot0 // 16:
                                      (slot0 + ntiles * 128) // 16])
                # section list for this run: (bb, t_lo, t_hi, kvar)
                secs = []
                for bb in cfg.blocks_of_super(s):
                    ln = int(Lsec[g, bb])
                    if ln == 0:
                        continue
                    o0 = cfg.sec_off[(g, bb)]
                    secs.append((bb, o0 // 128, (o0 + ln - 1) // 128,
                                 (bb - s * cfg.bs) % 4))
                nchunks = (ntiles + gchunk - 1) // gchunk
                msg_tiles = [None] * nchunks
                si = 0
                for c in range(nchunks):
                    c0 = c * gchunk
                    cnt = min(gchunk, ntiles - c0)
                    mt = msgp.tile([128, gchunk, 128], bf16, name="msg_ch",
                                   tag="msg")
                    msg_tiles[c] = mt
                    n_sg = cnt * 128
                    nc.gpsimd.dma_gather(
                        mt[:, :cnt, :],
                        cc_out[g][:, :],
                        idx_t[:, c0 * 8:c0 * 8 + n_sg // 16],
                        n_sg, n_sg, DO,
                        queue_num=gidx[0] % nq)
                    gidx[0] += 1
                    # sections whose last tile lands in this chunk: full
                    # selector + matmul chain + drain (msg read from the
                    # 1-2 covering chunk tiles, all still pool-live)
                    while si < len(secs) and secs[si][2] < c0 + cnt:
                        bb, t_lo, t_hi, kvar = secs[si]
                        si += 1
                        span = t_hi - t_lo + 1
                        sel = selp.tile([128, smax * 128], bf16, tag="sel")
                        s_view = sel[:, :span * 128].rearrange(
                            "p (t j) -> p t j", j=128)
                        d_ap = dst_sb[:, slot0 // 128 + t_lo:
                                      slot0 // 128 + t_hi + 1]
                        d_b = d_ap.unsqueeze(2).broadcast_to(
                            (128, span, 128))
                        i_b = iota4[:, kvar, :].unsqueeze(1).broadcast_to(
                            (128, span, 128))
                        nc.vector.tensor_tensor(s_view, d_b, i_b,
                                                mybir.AluOpType.is_equal)
                        ps = pspool.tile([128, DO], fp32, tag="ps")
                        for ti, t in enumerate(range(t_lo, t_hi + 1)):
                            nc.tensor.matmul(
                                ps[:, :], sel[:, ti * 128:(ti + 1) * 128],
                                msg_tiles[t // gchunk][:, t % gchunk, :],
                                start=(ti == 0), stop=(ti == span - 1))
                        # acc[:, bb, :] += dinv[bb] * ps
                        nc.vector.scalar_tensor_tensor(
                            acc[:, bb, :], ps[:, :], dinv[:, bb:bb + 1],
                            acc[:, bb, :],
                            mybir.AluOpType.mult, mybir.AluOpType.add)
                        if g == last_g[bb]:
                            t2 = epool.tile([128, DO], fp32, tag="t2")
                            nc.scalar.activation(
                                t2[:, :], acc[:, bb, :],
                                mybir.ActivationFunctionType.Relu)
                            rows = min(128, cfg.ns - bb * 128)
                            nc.sync.dma_start(
                                out[bb * 128:bb * 128 + rows, :],
                                t2[:rows, :])
                assert si == len(secs), (si, len(secs))
        nc.leave_named_scope("phC", _sidC, False)

    nc.compile()
    return nc


def kernel(x, edge_index, W, b):
    global LAST_RES
    cfg = Cfg(N_NODES, D_IN, D_OUT, N_CORES)
    in_maps, tot_slots = preprocess(x, edge_index, W, b, cfg)
    nc = build_program(cfg)
    res = run_bass_kernel_spmd(nc, in_maps, list(range(N_CORES)), trace=TRACE)
    LAST_RES = res
    outs = [r["out"][:cfg.ns] for r in res.results]
    return np.concatenate(outs, axis=0).astype(np.float32)


if __name__ == "__main__":
    cfg = Cfg(N_NODES, D_IN, D_OUT, N_CORES)
    print("cfg", cfg.nb, cfg.nsup, cfg.slice_blocks, cfg.group_rows)
